# revision 36
# baseline (speedup 1.0000x reference)
"""Trainium2 Bass kernel for nn_DeformableDynamicGather1D.

Sharding: 8 cores = 4 batches x 2 query-halves. Each core: one batch's
feat [256, 4096], Q=4096 queries.

Phases (software-pipelined per 1024-query chunk; Tile overlaps chunks):
  T. Transpose feat [C, L] -> feat_T [L, C] in DRAM (PE transposes, one
     staging buffer, ONE store DMA). feat stays resident in SBUF as two
     [128, 4096] halves for the anchor ap_gather.
  A. Anchor (per chunk): ap_gather f0/f1 channel-major directly from SBUF
     feat halves (GPSIMD), bilinear lerp with a PE-replicated frac row,
     writing rinT chunks. No PE transposes needed.
  M. MLP (per chunk): h = leaky(rin@W1+b1); g = leaky(h@(Wr+I)+br);
     out3 = [g;1]@[W3;b3] (residual folded into Wr+I, b3 via ones row).
  S. Scalar stage (per chunk, query-major [128, 8] tiles): softplus/clips,
     tanh, sigmoid, offsets, deform indices, normalized weights c0/c1.
  G. Deform (per chunk): dma_gather 5 taps of 2KB row-pairs (rows i0,i0+1,
     elem_step=256) query-major; FMA-combine split across DVE (stt) and
     ACT (nc.any ts+tt); per-chunk 1MB out DMA.

Query <-> tile coords: q = g*128 + p. dma_gather/ap_gather read index j
from a wrapped int16 tile at [j%16, j//16] (16-row block replicated on all
128 partitions for the 8 Q7 cores); dma_gather writes chunk j to
out [j%128, j//128]. With j = q the wrapped tile w[b, f] = i0(q=16f+b) is
built from the query-major f32 index tile V [128, nk*G] by 8 constant
selection matmuls W_a[m, n] = V[16a + m%16, n] (PE folds partitions and
replicates in one shot), strided int16-converting copies (col f = g*8+a).
"""
import os
import sys

for _p in ("/opt/trn_rl_repo", "/root/.axon_site/_ro/trn_rl_repo"):
    if os.path.isdir(_p) and _p not in sys.path:
        sys.path.append(_p)

import numpy as np
import concourse.bass as bass
import concourse.bacc as bacc
import concourse.tile as tile
from concourse import mybir
from concourse.bass import AP
from concourse.masks import make_identity

F32 = mybir.dt.float32
I16 = mybir.dt.int16
I32 = mybir.dt.int32
Act = mybir.ActivationFunctionType
Alu = mybir.AluOpType

P = 128          # partitions
G = 32           # q = g*128 + p
Q = P * G        # 4096 queries per core
C = 256          # channels
L = 4096         # feat length
H = 64           # hidden
K = 5            # taps
NCORES = 8
B, N = 4, 8192   # full problem
NI = 1024        # queries per pipeline chunk
NCH = Q // NI    # 4 chunks
GPC = NI // P    # 8 g-columns per chunk

IXSCALE = np.float32(float(L - 1))          # 4095
DXSCALE = np.float32(2.0 / max(L - 1, 1))   # reference scale_x

DEBUG_DUMPS = False


def _bc(ap2d: AP, extra: int) -> AP:
    return AP(tensor=ap2d.tensor, offset=ap2d.offset,
              ap=[*ap2d.ap, [0, extra]])


def _bc_mid(ap2d: AP, mid: int) -> AP:
    return AP(tensor=ap2d.tensor, offset=ap2d.offset,
              ap=[ap2d.ap[0], [0, mid], ap2d.ap[1]])


def build_program():
    nc = bacc.Bacc("TRN2", target_bir_lowering=False, debug=False,
                   num_devices=NCORES)

    feat = nc.dram_tensor("feat", [C, L], F32, kind="ExternalInput")
    coords = nc.dram_tensor("coords", [Q], F32, kind="ExternalInput")
    cellv = nc.dram_tensor("cellv", [Q], F32, kind="ExternalInput")
    w1a0 = nc.dram_tensor("w1a0", [128, H], F32, kind="ExternalInput")
    w1a1 = nc.dram_tensor("w1a1", [128, H], F32, kind="ExternalInput")
    wxc = nc.dram_tensor("wxc", [2, H], F32, kind="ExternalInput")
    b1c = nc.dram_tensor("b1c", [H, 1], F32, kind="ExternalInput")
    wr1 = nc.dram_tensor("wr1", [H, H], F32, kind="ExternalInput")
    brc = nc.dram_tensor("brc", [H, 1], F32, kind="ExternalInput")
    w3aug = nc.dram_tensor("w3aug", [H + 1, 12], F32, kind="ExternalInput")
    base128 = nc.dram_tensor("base128", [P, K], F32, kind="ExternalInput")
    sel8 = nc.dram_tensor("sel8", [P, 8 * 128], F32, kind="ExternalInput")
    colsel = nc.dram_tensor("colsel", [G, G * P], F32, kind="ExternalInput")
    out = nc.dram_tensor("out", [Q, C], F32, kind="ExternalOutput")

    dbg = {}
    if DEBUG_DUMPS:
        dbg = {
            "d_out3": nc.dram_tensor("d_out3", [P, G * 12], F32, kind="ExternalOutput"),
            "d_rin0": nc.dram_tensor("d_rin0", [P, NI], F32, kind="ExternalOutput"),
        }

    with tile.TileContext(nc) as tc:
        _body(nc, tc, feat, coords, cellv, w1a0, w1a1, wxc, b1c, wr1, brc,
              w3aug, base128, sel8, colsel, out, dbg)
    nc.compile()
    return nc


def _body(nc, tc, feat, coords, cellv, w1a0, w1a1, wxc, b1c, wr1, brc,
          w3aug, base128, sel8, colsel, out, dbg=None):
    dbg = dbg or {}
    import contextlib
    ctx = contextlib.ExitStack()
    with ctx:
        persist = ctx.enter_context(tc.tile_pool(name="persist", bufs=1))
        small = ctx.enter_context(tc.tile_pool(name="small", bufs=1))
        anc = ctx.enter_context(tc.tile_pool(name="anc", bufs=1))
        rinp = ctx.enter_context(tc.tile_pool(name="rinp", bufs=2))
        mlpp = ctx.enter_context(tc.tile_pool(name="mlpp", bufs=2))
        gath = ctx.enter_context(tc.tile_pool(name="gath", bufs=2))
        obp = ctx.enter_context(tc.tile_pool(name="obp", bufs=2))
        scp = ctx.enter_context(tc.tile_pool(name="scp", bufs=2))
        big32 = ctx.enter_context(tc.tile_pool(name="big32", bufs=1))
        pst = ctx.enter_context(tc.tile_pool(name="pst", bufs=2, space="PSUM"))
        psmm = ctx.enter_context(tc.tile_pool(name="psmm", bufs=2, space="PSUM"))
        psw = ctx.enter_context(tc.tile_pool(name="psw", bufs=2, space="PSUM"))
        dram = ctx.enter_context(tc.tile_pool(name="dram", bufs=1, space="DRAM"))

        ident = small.tile([P, P], F32)
        make_identity(nc, ident[:])
        ones1 = small.tile([1, P], F32)
        nc.vector.memset(ones1[:], 1.0)

        feat_T = dram.tile([L, C], F32)
        fsb0 = persist.tile([P, L], F32)   # feat channels 0..127
        fsb1 = persist.tile([P, L], F32)   # feat channels 128..255
        xc = persist.tile([2, Q], F32)
        out3 = persist.tile([P, G, 12], F32)

        # weights / constants
        w1a0_sb = small.tile([128, H], F32)
        w1a1_sb = small.tile([128, H], F32)
        wxc_sb = small.tile([2, H], F32)
        b1_sb = small.tile([H, 1], F32)
        wr1_sb = small.tile([H, H], F32)
        br_sb = small.tile([H, 1], F32)
        w3_sb = small.tile([H + 1, 12], F32)
        base_sb = small.tile([P, K], F32)
        sel_sb = small.tile([P, 8 * 128], F32)
        colsel_sb = small.tile([G, G * P], F32)
        for dst, src in ((w1a0_sb, w1a0), (w1a1_sb, w1a1), (wxc_sb, wxc),
                         (b1_sb, b1c), (wr1_sb, wr1), (br_sb, brc),
                         (w3_sb, w3aug), (base_sb, base128), (sel_sb, sel8),
                         (colsel_sb, colsel)):
            nc.sync.dma_start(out=dst[:], in_=src.ap())

        # feat_T row-pair view for dma_gather: row i = elems [256*i, 256*i+512)
        gsrc = AP(tensor=feat_T[:].tensor, offset=0,
                  ap=[[C, L - 1], [1, 2 * C]])

        def wrapped_idx(vf32_ap, nk, ncols, wrep, wcol0):
            """Wrapped int16 idx build into wrep[:, :, wcol0:wcol0+ncols*8]
            from query-major f32 V [128, ncols*nk] ((g, k) cols)."""
            for a in range(8):
                pw = psw.tile([P, 8 * K], F32, tag="pswrap", space="PSUM")
                nc.tensor.matmul(
                    out=pw[:, :ncols * nk],
                    lhsT=sel_sb[:, a * 128:(a + 1) * 128],
                    rhs=vf32_ap, start=True, stop=True)
                dst = AP(tensor=wrep[:].tensor,
                         offset=wrep[:].offset + wcol0 + a,
                         ap=[wrep[:].ap[0], [Q // 16, nk], [8, ncols]])
                src = AP(tensor=pw[:].tensor, offset=pw[:].offset,
                         ap=[pw[:].ap[0], [1, nk], [nk, ncols]])
                nc.vector.tensor_copy(out=dst, in_=src)

        # =========== Phase T: feat load + transpose to feat_T ===========
        nc.sync.dma_start(out=fsb0[:], in_=feat.ap()[0:128, :])
        nc.sync.dma_start(out=fsb1[:], in_=feat.ap()[128:256, :])
        for half in range(2):
            stag = big32.tile([P, G // 2, C], F32, tag="big32")
            for t2 in range(G // 2):
                t = half * (G // 2) + t2
                for hh, fsb in ((0, fsb0), (1, fsb1)):
                    tp = pst.tile([P, P], F32, tag="tpsum", space="PSUM")
                    nc.tensor.transpose(out=tp[:],
                                        in_=fsb[:, t * 128:(t + 1) * 128],
                                        identity=ident[:])
                    nc.scalar.copy(out=stag[:, t2, hh * 128:(hh + 1) * 128],
                                   in_=tp[:])
            nc.sync.dma_start(
                out=feat_T[half * (L // 2):(half + 1) * (L // 2), :]
                .rearrange("(t p) c -> p t c", p=P),
                in_=stag[:])

        # =========== anchor indices (all queries up front) ==========
        xq = persist.tile([P, G], F32)
        nc.sync.dma_start(
            out=xq[:],
            in_=AP(tensor=coords.ap().tensor, offset=0, ap=[[1, P], [P, G]]))
        nc.sync.dma_start(out=xc[0:1, :], in_=coords.ap().rearrange(
            "(a q) -> a q", a=1))
        nc.sync.dma_start(out=xc[1:2, :], in_=cellv.ap().rearrange(
            "(a q) -> a q", a=1))

        ixf = persist.tile([P, G], F32)
        nc.vector.tensor_scalar(out=ixf[:], in0=xq[:], scalar1=1.0,
                                scalar2=0.5, op0=Alu.add, op1=Alu.mult)
        nc.vector.tensor_scalar(out=ixf[:], in0=ixf[:], scalar1=float(IXSCALE),
                                scalar2=0.0, op0=Alu.mult, op1=Alu.max)
        nc.vector.tensor_scalar(out=ixf[:], in0=ixf[:], scalar1=float(IXSCALE),
                                scalar2=None, op0=Alu.min)
        fraca = persist.tile([P, G], F32)
        i0fa = small.tile([P, G], F32)
        ti_a = small.tile([P, G], I32)
        nc.vector.tensor_copy(out=ti_a[:], in_=ixf[:])
        nc.vector.tensor_copy(out=i0fa[:], in_=ti_a[:])
        gt_a = small.tile([P, G], F32)
        nc.vector.tensor_tensor(out=gt_a[:], in0=i0fa[:], in1=ixf[:],
                                op=Alu.is_gt)
        nc.vector.tensor_tensor(out=i0fa[:], in0=i0fa[:], in1=gt_a[:],
                                op=Alu.subtract)
        nc.vector.tensor_scalar(out=i0fa[:], in0=i0fa[:], scalar1=float(L - 2),
                                scalar2=None, op0=Alu.min)
        nc.vector.tensor_tensor(out=fraca[:], in0=ixf[:], in1=i0fa[:],
                                op=Alu.subtract)

        # fracT[g, pp] = frac(q = g*128+pp): one PE transpose
        fracT = persist.tile([G, P], F32)
        tpf = pst.tile([P, P], F32, tag="tpsum", space="PSUM")
        nc.tensor.transpose(out=tpf[:G, :], in_=fraca[:], identity=ident[:])
        nc.scalar.copy(out=fracT[:], in_=tpf[:G, :])

        wrapA = persist.tile([P, 1, Q // 16], I16)
        wrapped_idx(i0fa[:], 1, G, wrapA, 0)
        wrapA1 = persist.tile([P, 1, Q // 16], I16)
        nc.vector.tensor_scalar(out=wrapA1[:], in0=wrapA[:], scalar1=1,
                                scalar2=None, op0=Alu.add)
        wrapD = persist.tile([P, K, Q // 16], I16)

        # per-query coefficient tiles, filled per chunk by phase S
        c0 = persist.tile([P, G * K], F32)
        c1 = persist.tile([P, G * K], F32)

        for ch in range(NCH):
            csl = slice(ch * NI, (ch + 1) * NI)
            wsl = slice(ch * (NI // 16), (ch + 1) * (NI // 16))
            gsl = slice(ch * GPC, (ch + 1) * GPC)

            # ---- Phase A: anchor gather + lerp -> rin chunks ----
            rin0 = rinp.tile([P, NI], F32, tag="rin0")
            rin1 = rinp.tile([P, NI], F32, tag="rin1")
            # frac replicated across partitions for this chunk
            frep = anc.tile([P, NI], F32, tag="frep")
            for gi in range(GPC):
                g = ch * GPC + gi
                pf = pst.tile([P, P], F32, tag="tpsum", space="PSUM")
                nc.tensor.matmul(
                    out=pf[:], lhsT=colsel_sb[:, g * P:(g + 1) * P],
                    rhs=fracT[:], start=True, stop=True)
                nc.scalar.copy(out=frep[:, gi * 128:(gi + 1) * 128], in_=pf[:])
            for hh, fsb, rin in ((0, fsb0, rin0), (1, fsb1, rin1)):
                f0 = anc.tile([P, NI], F32, tag=f"f0_{hh}")
                f1 = anc.tile([P, NI], F32, tag=f"f1_{hh}")
                nc.gpsimd.ap_gather(
                    out_ap=f0[:], in_ap=fsb[:],
                    idxs_ap=wrapA[:, 0, wsl],
                    channels=P, num_elems=L, d=1, num_idxs=NI)
                nc.gpsimd.ap_gather(
                    out_ap=f1[:], in_ap=fsb[:],
                    idxs_ap=wrapA1[:, 0, wsl],
                    channels=P, num_elems=L, d=1, num_idxs=NI)
                nc.any.tensor_tensor(out=f1[:], in0=f1[:], in1=f0[:],
                                     op=Alu.subtract)
                nc.any.tensor_tensor(out=f1[:], in0=f1[:], in1=frep[:],
                                     op=Alu.mult)
                nc.any.tensor_tensor(out=rin[:], in0=f0[:], in1=f1[:],
                                     op=Alu.add)
            if ch == 0 and "d_rin0" in dbg:
                nc.sync.dma_start(out=dbg["d_rin0"].ap(), in_=rin0[:])

            # ---- Phase M: MLP for this chunk ----
            gaug = mlpp.tile([H + 1, NI], F32, tag="gaug")
            nc.vector.memset(gaug[H:H + 1, :], 1.0)
            for n2 in range(2):
                nsl = slice(n2 * 512, (n2 + 1) * 512)
                xsl = slice(ch * NI + n2 * 512, ch * NI + (n2 + 1) * 512)
                ps1 = psmm.tile([H, 512], F32, tag="ps1", space="PSUM")
                nc.tensor.matmul(out=ps1[:], lhsT=w1a0_sb[:], rhs=rin0[:, nsl],
                                 start=True, stop=False)
                nc.tensor.matmul(out=ps1[:], lhsT=w1a1_sb[:], rhs=rin1[:, nsl],
                                 start=False, stop=False)
                nc.tensor.matmul(out=ps1[:], lhsT=wxc_sb[:], rhs=xc[:, xsl],
                                 start=False, stop=True)
                tmp = mlpp.tile([H, 512], F32, tag="mlptmp")
                nc.scalar.activation(out=tmp[:], in_=ps1[:], func=Act.Identity,
                                     bias=b1_sb[:, :], scale=1.0)
                hck = mlpp.tile([H, 512], F32, tag="hck")
                nc.vector.scalar_tensor_tensor(out=hck[:], in0=tmp[:],
                                               scalar=0.2, in1=tmp[:],
                                               op0=Alu.mult, op1=Alu.max)
                ps2 = psmm.tile([H, 512], F32, tag="ps1", space="PSUM")
                nc.tensor.matmul(out=ps2[:], lhsT=wr1_sb[:], rhs=hck[:],
                                 start=True, stop=True)
                tmp2 = mlpp.tile([H, 512], F32, tag="mlptmp")
                nc.scalar.activation(out=tmp2[:], in_=ps2[:], func=Act.Identity,
                                     bias=br_sb[:, :], scale=1.0)
                nc.vector.scalar_tensor_tensor(out=gaug[0:H, nsl], in0=tmp2[:],
                                               scalar=0.2, in1=tmp2[:],
                                               op0=Alu.mult, op1=Alu.max)
            for gi in range(GPC):
                g = ch * GPC + gi
                ps3 = psw.tile([P, 12], F32, tag="pswrap", space="PSUM")
                nc.tensor.matmul(out=ps3[:],
                                 lhsT=gaug[:, gi * 128:(gi + 1) * 128],
                                 rhs=w3_sb[:], start=True, stop=True)
                nc.scalar.copy(out=out3[:, g, :], in_=ps3[:])

            # ---- Phase S: scalar stage for this chunk ----
            o3c = out3[:, gsl, :]   # [P, GPC, 12]

            def softplus(dst, src_ap, tag):
                a = scp.tile([P, GPC], F32, tag=tag + "_a")
                nc.scalar.activation(out=a[:], in_=src_ap, func=Act.Abs)
                e = scp.tile([P, GPC], F32, tag=tag + "_e")
                nc.scalar.activation(out=e[:], in_=a[:], func=Act.Exp,
                                     scale=-1.0)
                lg = scp.tile([P, GPC], F32, tag=tag + "_l")
                nc.scalar.activation(out=lg[:], in_=e[:], func=Act.Ln,
                                     bias=1.0, scale=1.0)
                m = scp.tile([P, GPC], F32, tag=tag + "_m")
                nc.vector.tensor_scalar(out=m[:], in0=src_ap, scalar1=0.0,
                                        scalar2=None, op0=Alu.max)
                nc.vector.tensor_tensor(out=dst, in0=lg[:], in1=m[:],
                                        op=Alu.add)

            r_t = scp.tile([P, GPC], F32, tag="r")
            softplus(r_t[:], o3c[:, :, 0], "spr")
            nc.vector.tensor_scalar(out=r_t[:], in0=r_t[:], scalar1=0.3,
                                    scalar2=2.0, op0=Alu.add, op1=Alu.min)
            sg_t = scp.tile([P, GPC], F32, tag="sg")
            softplus(sg_t[:], o3c[:, :, 1], "spg")
            nc.vector.tensor_scalar(out=sg_t[:], in0=sg_t[:], scalar1=0.5,
                                    scalar2=3.0, op0=Alu.add, op1=Alu.min)
            s2 = scp.tile([P, GPC], F32, tag="s2")
            nc.vector.tensor_tensor(out=s2[:], in0=sg_t[:], in1=sg_t[:],
                                    op=Alu.mult)
            nc.vector.tensor_scalar(out=s2[:], in0=s2[:], scalar1=4.0,
                                    scalar2=1e-8, op0=Alu.mult, op1=Alu.add)
            rs = scp.tile([P, GPC], F32, tag="rs")
            nc.vector.reciprocal(out=rs[:], in_=s2[:])

            NK = GPC * K
            res_t = scp.tile([P, NK], F32, tag="res")
            nc.scalar.activation(out=res_t[:], in_=o3c[:, :, 2:7],
                                 func=Act.Tanh)
            gate_t = scp.tile([P, NK], F32, tag="gate")
            nc.scalar.activation(out=gate_t[:], in_=o3c[:, :, 7:12],
                                 func=Act.Sigmoid)

            off_t = scp.tile([P, NK], F32, tag="off")
            nc.vector.tensor_tensor(out=off_t[:], in0=_bc(r_t[:], K),
                                    in1=_bc_mid(base_sb[:], GPC), op=Alu.mult)
            nc.vector.scalar_tensor_tensor(out=off_t[:], in0=res_t[:],
                                           scalar=0.5, in1=off_t[:],
                                           op0=Alu.mult, op1=Alu.add)
            dix = scp.tile([P, NK], F32, tag="dix")
            nc.vector.scalar_tensor_tensor(out=dix[:], in0=off_t[:],
                                           scalar=float(DXSCALE),
                                           in1=_bc(xq[:, gsl], K),
                                           op0=Alu.mult, op1=Alu.add)
            nc.vector.tensor_scalar(out=dix[:], in0=dix[:], scalar1=1.0,
                                    scalar2=0.5, op0=Alu.add, op1=Alu.mult)
            nc.vector.tensor_scalar(out=dix[:], in0=dix[:],
                                    scalar1=float(IXSCALE),
                                    scalar2=0.0, op0=Alu.mult, op1=Alu.max)
            nc.vector.tensor_scalar(out=dix[:], in0=dix[:],
                                    scalar1=float(IXSCALE),
                                    scalar2=None, op0=Alu.min)
            fracd = scp.tile([P, NK], F32, tag="fracd")
            i0fd = scp.tile([P, NK], F32, tag="i0fd")
            ti_d = scp.tile([P, NK], I32, tag="tid")
            nc.vector.tensor_copy(out=ti_d[:], in_=dix[:])
            nc.vector.tensor_copy(out=i0fd[:], in_=ti_d[:])
            gt_d = scp.tile([P, NK], F32, tag="gtd")
            nc.vector.tensor_tensor(out=gt_d[:], in0=i0fd[:], in1=dix[:],
                                    op=Alu.is_gt)
            nc.vector.tensor_tensor(out=i0fd[:], in0=i0fd[:], in1=gt_d[:],
                                    op=Alu.subtract)
            nc.vector.tensor_scalar(out=i0fd[:], in0=i0fd[:],
                                    scalar1=float(L - 2),
                                    scalar2=None, op0=Alu.min)
            nc.vector.tensor_tensor(out=fracd[:], in0=dix[:], in1=i0fd[:],
                                    op=Alu.subtract)

            o2 = scp.tile([P, NK], F32, tag="o2")
            nc.vector.tensor_tensor(out=o2[:], in0=off_t[:], in1=off_t[:],
                                    op=Alu.mult)
            nc.vector.tensor_tensor(out=o2[:], in0=o2[:], in1=_bc(rs[:], K),
                                    op=Alu.mult)
            w_t = scp.tile([P, NK], F32, tag="w")
            nc.scalar.activation(out=w_t[:], in_=o2[:], func=Act.Exp,
                                 scale=-0.5)
            nc.vector.tensor_tensor(out=w_t[:], in0=w_t[:], in1=gate_t[:],
                                    op=Alu.mult)
            wsum = scp.tile([P, GPC], F32, tag="wsum")
            w_v = w_t[:].rearrange("p (g k) -> p g k", k=K)
            nc.vector.tensor_reduce(out=wsum[:], in_=w_v,
                                    axis=mybir.AxisListType.X, op=Alu.add)
            nc.vector.tensor_scalar(out=wsum[:], in0=wsum[:], scalar1=1e-8,
                                    scalar2=None, op0=Alu.add)
            rn = scp.tile([P, GPC], F32, tag="rn")
            nc.vector.reciprocal(out=rn[:], in_=wsum[:])
            wn = scp.tile([P, NK], F32, tag="wn")
            nc.vector.tensor_tensor(out=wn[:], in0=w_t[:], in1=_bc(rn[:], K),
                                    op=Alu.mult)
            c0c = c0[:, ch * NK:(ch + 1) * NK]
            c1c = c1[:, ch * NK:(ch + 1) * NK]
            nc.vector.tensor_tensor(out=c1c, in0=wn[:], in1=fracd[:],
                                    op=Alu.mult)
            nc.vector.tensor_tensor(out=c0c, in0=wn[:], in1=c1c,
                                    op=Alu.subtract)

            wrapped_idx(i0fd[:], K, GPC, wrapD, ch * (NI // 16))

            # ---- Phase G: deform gathers + combine for this chunk ----
            ob = obp.tile([P, GPC, C], F32, tag="ob")
            for k in range(K):
                Gd = gath.tile([P, GPC, 2 * C], F32, tag="gath")
                nc.gpsimd.dma_gather(
                    out_ap=Gd[:], in_ap=gsrc,
                    idxs_ap=wrapD[:, k, wsl],
                    num_idxs=NI, num_idxs_reg=NI, elem_size=2 * C,
                    elem_step=C)
                for gi in range(GPC):
                    g = ch * GPC + gi
                    acc = ob[:, gi, :]
                    cc0 = c0[:, g * K + k:g * K + k + 1]
                    cc1 = c1[:, g * K + k:g * K + k + 1]
                    if k == 0:
                        nc.vector.tensor_scalar(
                            out=acc, in0=Gd[:, gi, 0:256],
                            scalar1=cc0, scalar2=None, op0=Alu.mult)
                        nc.vector.scalar_tensor_tensor(
                            out=acc, in0=Gd[:, gi, 256:512], scalar=cc1,
                            in1=acc, op0=Alu.mult, op1=Alu.add)
                    elif gi % 2 == 0:
                        nc.vector.scalar_tensor_tensor(
                            out=acc, in0=Gd[:, gi, 0:256], scalar=cc0,
                            in1=acc, op0=Alu.mult, op1=Alu.add)
                        nc.vector.scalar_tensor_tensor(
                            out=acc, in0=Gd[:, gi, 256:512], scalar=cc1,
                            in1=acc, op0=Alu.mult, op1=Alu.add)
                    else:
                        # route via nc.any (ACT picks these up when DVE busy)
                        m0 = obp.tile([P, C], F32, tag="accm")
                        nc.any.tensor_scalar(out=m0[:], in0=Gd[:, gi, 0:256],
                                             scalar1=cc0, scalar2=None,
                                             op0=Alu.mult)
                        nc.any.tensor_tensor(out=acc, in0=acc, in1=m0[:],
                                             op=Alu.add)
                        m1 = obp.tile([P, C], F32, tag="accm")
                        nc.any.tensor_scalar(out=m1[:], in0=Gd[:, gi, 256:512],
                                             scalar1=cc1, scalar2=None,
                                             op0=Alu.mult)
                        nc.any.tensor_tensor(out=acc, in0=acc, in1=m1[:],
                                             op=Alu.add)
            dstv = out.ap().rearrange("(g p) c -> p g c", p=P)[:, gsl, :]
            nc.sync.dma_start(out=dstv, in_=ob[:])

        if "d_out3" in dbg:
            nc.sync.dma_start(out=dbg["d_out3"].ap(), in_=out3[:])


_PROGRAM = None


def _get_program():
    global _PROGRAM
    if _PROGRAM is None:
        _PROGRAM = build_program()
    return _PROGRAM


def make_in_maps(feat_1d, coords_1d, cell_1d, W1, b1, Wr, br, W3, b3):
    f32 = np.float32
    W1 = np.asarray(W1, f32)
    wr1 = np.asarray(Wr, f32) + np.eye(H, dtype=f32)
    w3aug = np.concatenate([np.asarray(W3, f32),
                            np.asarray(b3, f32).reshape(1, 12)], axis=0)
    base = np.array([-2.0, -1.0, 0.0, 1.0, 2.0], f32)
    base128 = np.broadcast_to(base, (P, K)).copy()
    sel = np.zeros((P, 8, 128), f32)
    for a in range(8):
        for m in range(128):
            sel[16 * a + m % 16, a, m] = 1.0
    shared = {
        "w1a0": np.ascontiguousarray(W1[0:128]),
        "w1a1": np.ascontiguousarray(W1[128:256]),
        "wxc": np.ascontiguousarray(W1[256:258]),
        "b1c": np.asarray(b1, f32).reshape(H, 1).copy(),
        "wr1": wr1,
        "brc": np.asarray(br, f32).reshape(H, 1).copy(),
        "w3aug": w3aug,
        "base128": base128,
        "sel8": sel.reshape(P, 8 * 128),
        "colsel": np.ascontiguousarray(
            np.transpose(np.eye(G, dtype=f32)[:, :, None] *
                         np.ones((1, 1, P), f32), (0, 1, 2)).reshape(G, G * P)),
    }
    in_maps = []
    for core in range(NCORES):
        b = core // 2
        s = core % 2
        sl = slice(s * Q, (s + 1) * Q)
        in_maps.append({
            "feat": np.ascontiguousarray(np.asarray(feat_1d[b], f32)),
            "coords": np.ascontiguousarray(np.asarray(coords_1d[b, sl, 0], f32)),
            "cellv": np.ascontiguousarray(np.asarray(cell_1d[b, sl, 0], f32)),
            **shared,
        })
    return in_maps


def kernel(feat_1d, coords_1d, cell_1d, W1, b1, Wr, br, W3, b3):
    from concourse.bass_utils import run_bass_kernel_spmd
    nc = _get_program()
    in_maps = make_in_maps(feat_1d, coords_1d, cell_1d, W1, b1, Wr, br, W3, b3)
    res = run_bass_kernel_spmd(nc, in_maps, core_ids=list(range(NCORES)))
    outf = np.zeros((B, N, C), np.float32)
    for core in range(NCORES):
        b = core // 2
        s = core % 2
        outf[b, s * Q:(s + 1) * Q, :] = res.results[core]["out"]
    return outf


# revision 41
# speedup vs baseline: 1.5251x; 1.5251x over previous
"""Trainium2 Bass kernel for nn_DeformableDynamicGather1D.

Sharding: 8 cores = 4 batches x 2 query-halves. Each core handles one batch's
feat [256, 4096] and Q=4096 queries. Per core:

  1. Transpose feat [C, L] -> feat_T [L, C] in DRAM (PE transposes, one
     staging buffer, ONE store DMA so downstream gathers have few sem waits).
  2. Anchor: bilinear indices from coords; dma_gather 2KB row-pairs
     (rows i0, i0+1 = 512 floats, elem_step=256) query-major; lerp on DVE;
     PE-transpose into channel-major rinT for the MLP.
  3. MLP on PE: h = leaky(rin@W1+b1); g = leaky(h@(Wr+I)+br);
     out3 = [g;1]@[W3;b3] per 128-query chunk (residual folded into Wr+I,
     b3 folded via augmented ones row).
  4. Scalar stage (query-major [128, 32] tiles): softplus/clips, tanh,
     sigmoid, offsets, deform indices, normalized bilinear weights c0/c1.
  5. Deform: dma_gather 5 taps x 4 chunks; accumulate with
     scalar_tensor_tensor FMAs into ob [128, 32, 256]; one 4MB out DMA.

Query <-> tile coordinates: q = g*128 + p (tile [128 p, 32 g]); dma_gather
places index-list position j at out [j%128, j//128] and reads idx j from a
wrapped int16 tile at [j%16, j//16] (16-row block replicated on all 128
partitions for the 8 Q7 cores). With j = q, the wrapped tile w[b, f] =
i0(q=16f+b) is built from the query-major f32 index tile V [128, (g,k)] by
8 constant selection matmuls W_a[b, n] = V[16a+b, n] (PE does the partition
fold), strided copies (col f = g*8 + a), int16 convert, and one 8x partition
replication DMA.
"""
import os
import sys

for _p in ("/opt/trn_rl_repo", "/root/.axon_site/_ro/trn_rl_repo"):
    if os.path.isdir(_p) and _p not in sys.path:
        sys.path.append(_p)

import numpy as np
import concourse.bass as bass
import concourse.bacc as bacc
import concourse.tile as tile
from concourse import mybir
from concourse.bass import AP
from concourse.masks import make_identity

F32 = mybir.dt.float32
I16 = mybir.dt.int16
I32 = mybir.dt.int32
Act = mybir.ActivationFunctionType
Alu = mybir.AluOpType

P = 128          # partitions
G = 32           # q = g*128 + p
Q = P * G        # 4096 queries per core
C = 256          # channels
L = 4096         # feat length
H = 64           # hidden
K = 5            # taps
NCORES = 8
B, N = 4, 8192   # full problem
NI = 1024        # idxs per dma_gather call
NCH = Q // NI    # 4 chunks
GPC = NI // P    # 8 g-columns per chunk

IXSCALE = np.float32(float(L - 1))          # 4095
DXSCALE = np.float32(2.0 / max(L - 1, 1))   # reference scale_x

DEBUG_DUMPS = False


def _bc(ap2d: AP, extra: int) -> AP:
    """Broadcast a [p, n] AP to [p, n, extra] with stride-0 inner dim."""
    return AP(tensor=ap2d.tensor, offset=ap2d.offset,
              ap=[*ap2d.ap, [0, extra]])


def _bc_mid(ap2d: AP, mid: int) -> AP:
    """Broadcast a [p, n] AP to [p, mid, n] with stride-0 middle dim."""
    return AP(tensor=ap2d.tensor, offset=ap2d.offset,
              ap=[ap2d.ap[0], [0, mid], ap2d.ap[1]])


def build_program():
    nc = bacc.Bacc("TRN2", target_bir_lowering=False, debug=False,
                   num_devices=NCORES)

    feat = nc.dram_tensor("feat", [C, L], F32, kind="ExternalInput")
    coords = nc.dram_tensor("coords", [Q], F32, kind="ExternalInput")
    cellv = nc.dram_tensor("cellv", [Q], F32, kind="ExternalInput")
    w1a0 = nc.dram_tensor("w1a0", [128, H], F32, kind="ExternalInput")
    w1a1 = nc.dram_tensor("w1a1", [128, H], F32, kind="ExternalInput")
    wxc = nc.dram_tensor("wxc", [2, H], F32, kind="ExternalInput")
    b1c = nc.dram_tensor("b1c", [H, 1], F32, kind="ExternalInput")
    wr1 = nc.dram_tensor("wr1", [H, H], F32, kind="ExternalInput")
    brc = nc.dram_tensor("brc", [H, 1], F32, kind="ExternalInput")
    w3aug = nc.dram_tensor("w3aug", [H + 1, 12], F32, kind="ExternalInput")
    base128 = nc.dram_tensor("base128", [P, K], F32, kind="ExternalInput")
    sel8 = nc.dram_tensor("sel8", [P, 8 * 128], F32, kind="ExternalInput")
    out = nc.dram_tensor("out", [Q, C], F32, kind="ExternalOutput")

    dbg = {}
    if DEBUG_DUMPS:
        dbg = {
            "d_featT": nc.dram_tensor("d_featT", [L, C], F32, kind="ExternalOutput"),
            "d_aidx": nc.dram_tensor("d_aidx", [P, G], F32, kind="ExternalOutput"),
            "d_wrapA": nc.dram_tensor("d_wrapA", [P, Q // 16], I16, kind="ExternalOutput"),
            "d_Ga0": nc.dram_tensor("d_Ga0", [P, GPC * 512], F32, kind="ExternalOutput"),
            "d_rinT0": nc.dram_tensor("d_rinT0", [P, Q], F32, kind="ExternalOutput"),
            "d_out3": nc.dram_tensor("d_out3", [P, G * 12], F32, kind="ExternalOutput"),
            "d_didx": nc.dram_tensor("d_didx", [P, G * K], F32, kind="ExternalOutput"),
            "d_c0": nc.dram_tensor("d_c0", [P, G * K], F32, kind="ExternalOutput"),
            "d_c1": nc.dram_tensor("d_c1", [P, G * K], F32, kind="ExternalOutput"),
            "d_Gd0": nc.dram_tensor("d_Gd0", [P, GPC * 512], F32, kind="ExternalOutput"),
        }

    with tile.TileContext(nc) as tc:
        _body(nc, tc, feat, coords, cellv, w1a0, w1a1, wxc, b1c, wr1, brc,
              w3aug, base128, sel8, out, dbg)
    nc.compile()
    return nc


def _body(nc, tc, feat, coords, cellv, w1a0, w1a1, wxc, b1c, wr1, brc,
          w3aug, base128, sel8, out, dbg=None):
    dbg = dbg or {}
    import contextlib
    ctx = contextlib.ExitStack()
    with ctx:
        persist = ctx.enter_context(tc.tile_pool(name="persist", bufs=1))
        small = ctx.enter_context(tc.tile_pool(name="small", bufs=1))
        tbuf = ctx.enter_context(tc.tile_pool(name="tbuf", bufs=2))
        gath = ctx.enter_context(tc.tile_pool(name="gath", bufs=2))
        fabuf = ctx.enter_context(tc.tile_pool(name="fabuf", bufs=2))
        big32 = ctx.enter_context(tc.tile_pool(name="big32", bufs=1))
        pst = ctx.enter_context(tc.tile_pool(name="pst", bufs=2, space="PSUM"))
        psmm = ctx.enter_context(tc.tile_pool(name="psmm", bufs=2, space="PSUM"))
        psl3 = ctx.enter_context(tc.tile_pool(name="psl3", bufs=2, space="PSUM"))
        dram = ctx.enter_context(tc.tile_pool(name="dram", bufs=1, space="DRAM"))

        ident = small.tile([P, P], F32)
        make_identity(nc, ident[:])

        feat_T = dram.tile([L, C], F32)
        rinT0 = persist.tile([P, Q], F32)      # channels 0..127, col = q
        rinT1 = persist.tile([P, Q], F32)      # channels 128..255
        xc = persist.tile([2, Q], F32)         # rows: coords, cell (q-contig)
        h_sb = persist.tile([H, Q], F32)
        gaug = persist.tile([H + 1, Q], F32)   # row H = 1.0 (b3 fold)
        out3 = persist.tile([P, G, 12], F32)

        # weights / constants
        w1a0_sb = small.tile([128, H], F32)
        w1a1_sb = small.tile([128, H], F32)
        wxc_sb = small.tile([2, H], F32)
        b1_sb = small.tile([H, 1], F32)
        wr1_sb = small.tile([H, H], F32)
        br_sb = small.tile([H, 1], F32)
        w3_sb = small.tile([H + 1, 12], F32)
        base_sb = small.tile([P, K], F32)
        sel_sb = small.tile([P, 8 * 128], F32)
        for dst, src in ((w1a0_sb, w1a0), (w1a1_sb, w1a1), (wxc_sb, wxc),
                         (b1_sb, b1c), (wr1_sb, wr1), (br_sb, brc),
                         (w3_sb, w3aug), (base_sb, base128), (sel_sb, sel8)):
            nc.sync.dma_start(out=dst[:], in_=src.ap())

        # feat_T row-pair view for dma_gather: row i = elems [256*i, 256*i+512)
        gsrc = AP(tensor=feat_T[:].tensor, offset=0,
                  ap=[[C, L - 1], [1, 2 * C]])

        def wrapped_idx(vf32_ap, nk, ncols, wrep, wcol0):
            """Wrapped int16 idx build into wrep cols [wcol0, wcol0+ncols*8)
            from query-major f32 V [128, ncols*nk] ((g, k) cols).
            SEL_a[pp, m] = (pp == 16a + m%16) -> W_a[m, n] = V[16a+m%16, n]
            (partition fold + 8x replication in one matmul)."""
            for a in range(8):
                psw = psl3.tile([P, GPC * K], F32, tag="pswrap", space="PSUM")
                nc.tensor.matmul(
                    out=psw[:, :ncols * nk],
                    lhsT=sel_sb[:, a * 128:(a + 1) * 128],
                    rhs=vf32_ap, start=True, stop=True)
                dst = AP(tensor=wrep[:].tensor,
                         offset=wrep[:].offset + wcol0 + a,
                         ap=[wrep[:].ap[0], [Q // 16, nk], [8, ncols]])
                srcp = AP(tensor=psw[:].tensor, offset=psw[:].offset,
                          ap=[psw[:].ap[0], [1, nk], [nk, ncols]])
                nc.vector.tensor_copy(out=dst, in_=srcp)

        # =========== Phase T: feat [C, L] -> feat_T [L, C] ===========
        for half in range(2):
            stag = big32.tile([P, G // 2, C], F32, tag="big32")
            for t2 in range(G // 2):
                t8h = half * (G // 2) + t2
                t8, s8 = t8h // 4, t8h % 4
                ft = tbuf.tile([P, 128], F32, tag="ftin")
                nc.sync.dma_start(
                    out=ft[:],
                    in_=feat.ap()[0:128, t8h * 128:(t8h + 1) * 128])
                ft2 = tbuf.tile([P, 128], F32, tag="ftin")
                nc.sync.dma_start(
                    out=ft2[:],
                    in_=feat.ap()[128:256, t8h * 128:(t8h + 1) * 128])
                tp = pst.tile([P, P], F32, tag="tpsum", space="PSUM")
                nc.tensor.transpose(out=tp[:], in_=ft[:],
                                    identity=ident[:])
                nc.scalar.copy(out=stag[:, t2, 0:128], in_=tp[:])
                tp2 = pst.tile([P, P], F32, tag="tpsum", space="PSUM")
                nc.tensor.transpose(out=tp2[:], in_=ft2[:],
                                    identity=ident[:])
                nc.scalar.copy(out=stag[:, t2, 128:256], in_=tp2[:])
            nc.sync.dma_start(
                out=feat_T[half * (L // 2):(half + 1) * (L // 2), :]
                .rearrange("(t p) c -> p t c", p=P),
                in_=stag[:])

        # =========== Phase A: coords, anchor idx, gather, rinT ==========
        # xq[p, g] = coords[g*128 + p]
        xq = persist.tile([P, G], F32)
        nc.sync.dma_start(
            out=xq[:],
            in_=AP(tensor=coords.ap().tensor, offset=0, ap=[[1, P], [P, G]]))
        nc.sync.dma_start(out=xc[0:1, :], in_=coords.ap().rearrange(
            "(a q) -> a q", a=1))
        nc.sync.dma_start(out=xc[1:2, :], in_=cellv.ap().rearrange(
            "(a q) -> a q", a=1))

        # ix = clip(((x + 1) * 0.5) * (L-1), 0, L-1)  (same op order as ref)
        ixf = persist.tile([P, G], F32)
        nc.vector.tensor_scalar(out=ixf[:], in0=xq[:], scalar1=1.0,
                                scalar2=0.5, op0=Alu.add, op1=Alu.mult)
        nc.vector.tensor_scalar(out=ixf[:], in0=ixf[:], scalar1=float(IXSCALE),
                                scalar2=0.0, op0=Alu.mult, op1=Alu.max)
        nc.vector.tensor_scalar(out=ixf[:], in0=ixf[:], scalar1=float(IXSCALE),
                                scalar2=None, op0=Alu.min)
        # i0 = min(floor(ix), L-2); frac = ix - i0 (identical bilinear result;
        # floor via int-convert + fixup, works for trunc or round-nearest)
        fraca = persist.tile([P, G], F32)
        i0fa = small.tile([P, G], F32)
        ti_a = small.tile([P, G], I32)
        nc.vector.tensor_copy(out=ti_a[:], in_=ixf[:])
        nc.vector.tensor_copy(out=i0fa[:], in_=ti_a[:])
        gt_a = small.tile([P, G], F32)
        nc.vector.tensor_tensor(out=gt_a[:], in0=i0fa[:], in1=ixf[:],
                                op=Alu.is_gt)
        nc.vector.tensor_tensor(out=i0fa[:], in0=i0fa[:], in1=gt_a[:],
                                op=Alu.subtract)
        nc.vector.tensor_scalar(out=i0fa[:], in0=i0fa[:], scalar1=float(L - 2),
                                scalar2=None, op0=Alu.min)
        nc.vector.tensor_tensor(out=fraca[:], in0=ixf[:], in1=i0fa[:],
                                op=Alu.subtract)
        if "d_aidx" in dbg:
            nc.sync.dma_start(out=dbg["d_aidx"].ap(), in_=i0fa[:])

        wrapA = persist.tile([P, 1, Q // 16], I16)
        wrapped_idx(i0fa[:], 1, G, wrapA, 0)
        if "d_wrapA" in dbg:
            nc.sync.dma_start(out=dbg["d_wrapA"].ap(), in_=wrapA[:])

        for ch in range(NCH):
            Ga = gath.tile([P, GPC, 2 * C], F32, tag="gath")
            nc.gpsimd.dma_gather(
                out_ap=Ga[:], in_ap=gsrc,
                idxs_ap=wrapA[:, 0, ch * (NI // 16):(ch + 1) * (NI // 16)],
                num_idxs=NI, num_idxs_reg=NI, elem_size=2 * C, elem_step=C)
            if ch == 0 and "d_Ga0" in dbg:
                nc.sync.dma_start(out=dbg["d_Ga0"].ap(), in_=Ga[:])
            for gi in range(GPC):
                g = ch * GPC + gi
                d = fabuf.tile([P, C], F32, tag="dlerp")
                nc.vector.tensor_tensor(out=d[:], in0=Ga[:, gi, 256:512],
                                        in1=Ga[:, gi, 0:256], op=Alu.subtract)
                fa = fabuf.tile([P, C], F32, tag="fa")
                nc.vector.scalar_tensor_tensor(
                    out=fa[:], in0=d[:], scalar=fraca[:, g:g + 1],
                    in1=Ga[:, gi, 0:256], op0=Alu.mult, op1=Alu.add)
                for hh in range(2):
                    tpa = pst.tile([P, P], F32, tag="tpsum", space="PSUM")
                    nc.tensor.transpose(out=tpa[:],
                                        in_=fa[:, hh * 128:(hh + 1) * 128],
                                        identity=ident[:])
                    rdst = (rinT0 if hh == 0 else rinT1)
                    nc.scalar.copy(out=rdst[:, g * 128:(g + 1) * 128],
                                   in_=tpa[:])
        if "d_rinT0" in dbg:
            nc.sync.dma_start(out=dbg["d_rinT0"].ap(), in_=rinT0[:])

        # =========== Phase M: MLP ===========
        nc.vector.memset(gaug[H:H + 1, :], 1.0)
        for n in range(8):
            sl = slice(n * 512, (n + 1) * 512)
            ps1 = psmm.tile([H, 512], F32, tag="ps1", space="PSUM")
            nc.tensor.matmul(out=ps1[:], lhsT=w1a0_sb[:], rhs=rinT0[:, sl],
                             start=True, stop=False)
            nc.tensor.matmul(out=ps1[:], lhsT=w1a1_sb[:], rhs=rinT1[:, sl],
                             start=False, stop=False)
            nc.tensor.matmul(out=ps1[:], lhsT=wxc_sb[:], rhs=xc[:, sl],
                             start=False, stop=True)
            tmp = fabuf.tile([H, 512], F32, tag="mlptmp")
            nc.scalar.activation(out=tmp[:], in_=ps1[:], func=Act.Identity,
                                 bias=b1_sb[:, :], scale=1.0)
            nc.vector.scalar_tensor_tensor(out=h_sb[:, sl], in0=tmp[:],
                                           scalar=0.2, in1=tmp[:],
                                           op0=Alu.mult, op1=Alu.max)
        for n in range(8):
            sl = slice(n * 512, (n + 1) * 512)
            ps2 = psmm.tile([H, 512], F32, tag="ps1", space="PSUM")
            nc.tensor.matmul(out=ps2[:], lhsT=wr1_sb[:], rhs=h_sb[:, sl],
                             start=True, stop=True)
            tmp2 = fabuf.tile([H, 512], F32, tag="mlptmp")
            nc.scalar.activation(out=tmp2[:], in_=ps2[:], func=Act.Identity,
                                 bias=br_sb[:, :], scale=1.0)
            nc.vector.scalar_tensor_tensor(out=gaug[0:H, sl], in0=tmp2[:],
                                           scalar=0.2, in1=tmp2[:],
                                           op0=Alu.mult, op1=Alu.max)
        for g in range(G):
            ps3 = psl3.tile([P, 12], F32, tag="ps3", space="PSUM")
            nc.tensor.matmul(out=ps3[:], lhsT=gaug[:, g * 128:(g + 1) * 128],
                             rhs=w3_sb[:], start=True, stop=True)
            nc.scalar.copy(out=out3[:, g, :], in_=ps3[:])
        if "d_out3" in dbg:
            nc.sync.dma_start(out=dbg["d_out3"].ap(), in_=out3[:])

        # =========== Phase S: scalar stage ===========
        sc = ctx.enter_context(tc.tile_pool(name="scal", bufs=1))

        def softplus(dst, src_ap):
            a = sc.tile([P, G], F32, tag="sp_a")
            nc.scalar.activation(out=a[:], in_=src_ap, func=Act.Abs)
            e = sc.tile([P, G], F32, tag="sp_e")
            nc.scalar.activation(out=e[:], in_=a[:], func=Act.Exp, scale=-1.0)
            lg = sc.tile([P, G], F32, tag="sp_l")
            nc.scalar.activation(out=lg[:], in_=e[:], func=Act.Ln, bias=1.0,
                                 scale=1.0)
            m = sc.tile([P, G], F32, tag="sp_m")
            nc.vector.tensor_scalar(out=m[:], in0=src_ap, scalar1=0.0,
                                    scalar2=None, op0=Alu.max)
            nc.vector.tensor_tensor(out=dst, in0=lg[:], in1=m[:], op=Alu.add)

        r_t = sc.tile([P, G], F32)
        softplus(r_t[:], out3[:, :, 0])
        nc.vector.tensor_scalar(out=r_t[:], in0=r_t[:], scalar1=0.3,
                                scalar2=2.0, op0=Alu.add, op1=Alu.min)
        sg_t = sc.tile([P, G], F32)
        softplus(sg_t[:], out3[:, :, 1])
        nc.vector.tensor_scalar(out=sg_t[:], in0=sg_t[:], scalar1=0.5,
                                scalar2=3.0, op0=Alu.add, op1=Alu.min)
        s2 = sc.tile([P, G], F32)
        nc.vector.tensor_tensor(out=s2[:], in0=sg_t[:], in1=sg_t[:],
                                op=Alu.mult)
        nc.vector.tensor_scalar(out=s2[:], in0=s2[:], scalar1=4.0,
                                scalar2=1e-8, op0=Alu.mult, op1=Alu.add)
        rs = sc.tile([P, G], F32)
        nc.vector.reciprocal(out=rs[:], in_=s2[:])

        res_t = sc.tile([P, G * K], F32)
        nc.scalar.activation(out=res_t[:], in_=out3[:, :, 2:7], func=Act.Tanh)
        gate_t = sc.tile([P, G * K], F32)
        nc.scalar.activation(out=gate_t[:], in_=out3[:, :, 7:12],
                             func=Act.Sigmoid)

        off_t = sc.tile([P, G * K], F32)
        nc.vector.tensor_tensor(out=off_t[:], in0=_bc(r_t[:], K),
                                in1=_bc_mid(base_sb[:], G), op=Alu.mult)
        nc.vector.scalar_tensor_tensor(out=off_t[:], in0=res_t[:], scalar=0.5,
                                       in1=off_t[:], op0=Alu.mult, op1=Alu.add)
        dix = sc.tile([P, G * K], F32)
        nc.vector.scalar_tensor_tensor(out=dix[:], in0=off_t[:],
                                       scalar=float(DXSCALE),
                                       in1=_bc(xq[:], K),
                                       op0=Alu.mult, op1=Alu.add)
        nc.vector.tensor_scalar(out=dix[:], in0=dix[:], scalar1=1.0,
                                scalar2=0.5, op0=Alu.add, op1=Alu.mult)
        nc.vector.tensor_scalar(out=dix[:], in0=dix[:], scalar1=float(IXSCALE),
                                scalar2=0.0, op0=Alu.mult, op1=Alu.max)
        nc.vector.tensor_scalar(out=dix[:], in0=dix[:], scalar1=float(IXSCALE),
                                scalar2=None, op0=Alu.min)
        fracd = sc.tile([P, G * K], F32)
        i0fd = sc.tile([P, G * K], F32)
        ti_d = sc.tile([P, G * K], I32)
        nc.vector.tensor_copy(out=ti_d[:], in_=dix[:])
        nc.vector.tensor_copy(out=i0fd[:], in_=ti_d[:])
        gt_d = sc.tile([P, G * K], F32)
        nc.vector.tensor_tensor(out=gt_d[:], in0=i0fd[:], in1=dix[:],
                                op=Alu.is_gt)
        nc.vector.tensor_tensor(out=i0fd[:], in0=i0fd[:], in1=gt_d[:],
                                op=Alu.subtract)
        nc.vector.tensor_scalar(out=i0fd[:], in0=i0fd[:], scalar1=float(L - 2),
                                scalar2=None, op0=Alu.min)
        nc.vector.tensor_tensor(out=fracd[:], in0=dix[:], in1=i0fd[:],
                                op=Alu.subtract)

        o2 = sc.tile([P, G * K], F32)
        nc.vector.tensor_tensor(out=o2[:], in0=off_t[:], in1=off_t[:],
                                op=Alu.mult)
        nc.vector.tensor_tensor(out=o2[:], in0=o2[:], in1=_bc(rs[:], K),
                                op=Alu.mult)
        w_t = sc.tile([P, G * K], F32)
        nc.scalar.activation(out=w_t[:], in_=o2[:], func=Act.Exp, scale=-0.5)
        nc.vector.tensor_tensor(out=w_t[:], in0=w_t[:], in1=gate_t[:],
                                op=Alu.mult)
        wsum = sc.tile([P, G], F32)
        w_v = w_t[:].rearrange("p (g k) -> p g k", k=K)
        nc.vector.tensor_reduce(out=wsum[:], in_=w_v, axis=mybir.AxisListType.X,
                                op=Alu.add)
        nc.vector.tensor_scalar(out=wsum[:], in0=wsum[:], scalar1=1e-8,
                                scalar2=None, op0=Alu.add)
        rn = sc.tile([P, G], F32)
        nc.vector.reciprocal(out=rn[:], in_=wsum[:])
        wn = sc.tile([P, G * K], F32)
        nc.vector.tensor_tensor(out=wn[:], in0=w_t[:], in1=_bc(rn[:], K),
                                op=Alu.mult)
        c1 = sc.tile([P, G * K], F32)
        nc.vector.tensor_tensor(out=c1[:], in0=wn[:], in1=fracd[:],
                                op=Alu.mult)
        c0 = sc.tile([P, G * K], F32)
        nc.vector.tensor_tensor(out=c0[:], in0=wn[:], in1=c1[:],
                                op=Alu.subtract)
        if "d_didx" in dbg:
            nc.sync.dma_start(out=dbg["d_didx"].ap(), in_=i0fd[:])
            nc.sync.dma_start(out=dbg["d_c0"].ap(), in_=c0[:])
            nc.sync.dma_start(out=dbg["d_c1"].ap(), in_=c1[:])

        wrapD = wrapped_idx(i0fd[:], K, "wd")

        # =========== Phase G: deform gather + combine + out ===========
        ob = big32.tile([P, G, C], F32, tag="big32")
        for k in range(K):
            for ch in range(NCH):
                Gd = gath.tile([P, GPC, 2 * C], F32, tag="gath")
                nc.gpsimd.dma_gather(
                    out_ap=Gd[:], in_ap=gsrc,
                    idxs_ap=wrapD[:, k, ch * (NI // 16):(ch + 1) * (NI // 16)],
                    num_idxs=NI, num_idxs_reg=NI, elem_size=2 * C, elem_step=C)
                if k == 0 and ch == 0 and "d_Gd0" in dbg:
                    nc.sync.dma_start(out=dbg["d_Gd0"].ap(), in_=Gd[:])
                for gi in range(GPC):
                    g = ch * GPC + gi
                    acc = ob[:, g, :]
                    if k == 0:
                        nc.vector.tensor_scalar(
                            out=acc, in0=Gd[:, gi, 0:256],
                            scalar1=c0[:, g * K + k:g * K + k + 1],
                            scalar2=None, op0=Alu.mult)
                    else:
                        nc.vector.scalar_tensor_tensor(
                            out=acc, in0=Gd[:, gi, 0:256],
                            scalar=c0[:, g * K + k:g * K + k + 1],
                            in1=acc, op0=Alu.mult, op1=Alu.add)
                    nc.vector.scalar_tensor_tensor(
                        out=acc, in0=Gd[:, gi, 256:512],
                        scalar=c1[:, g * K + k:g * K + k + 1],
                        in1=acc, op0=Alu.mult, op1=Alu.add)
        nc.sync.dma_start(
            out=out.ap().rearrange("(g p) c -> p g c", p=P), in_=ob[:])


_PROGRAM = None


def _get_program():
    global _PROGRAM
    if _PROGRAM is None:
        _PROGRAM = build_program()
    return _PROGRAM


def make_in_maps(feat_1d, coords_1d, cell_1d, W1, b1, Wr, br, W3, b3):
    """Build the 8 per-core input dicts from full inputs."""
    f32 = np.float32
    W1 = np.asarray(W1, f32)
    wr1 = np.asarray(Wr, f32) + np.eye(H, dtype=f32)
    w3aug = np.concatenate([np.asarray(W3, f32),
                            np.asarray(b3, f32).reshape(1, 12)], axis=0)
    base = np.array([-2.0, -1.0, 0.0, 1.0, 2.0], f32)
    base128 = np.broadcast_to(base, (P, K)).copy()
    sel = np.zeros((P, 8, 128), f32)
    for a in range(8):
        for m in range(128):
            sel[16 * a + m % 16, a, m] = 1.0
    shared = {
        "w1a0": np.ascontiguousarray(W1[0:128]),
        "w1a1": np.ascontiguousarray(W1[128:256]),
        "wxc": np.ascontiguousarray(W1[256:258]),
        "b1c": np.asarray(b1, f32).reshape(H, 1).copy(),
        "wr1": wr1,
        "brc": np.asarray(br, f32).reshape(H, 1).copy(),
        "w3aug": w3aug,
        "base128": base128,
        "sel8": sel.reshape(P, 8 * 128),
    }
    in_maps = []
    for core in range(NCORES):
        b = core // 2
        s = core % 2
        sl = slice(s * Q, (s + 1) * Q)
        in_maps.append({
            "feat": np.ascontiguousarray(np.asarray(feat_1d[b], f32)),
            "coords": np.ascontiguousarray(np.asarray(coords_1d[b, sl, 0], f32)),
            "cellv": np.ascontiguousarray(np.asarray(cell_1d[b, sl, 0], f32)),
            **shared,
        })
    return in_maps


def kernel(feat_1d, coords_1d, cell_1d, W1, b1, Wr, br, W3, b3):
    from concourse.bass_utils import run_bass_kernel_spmd
    nc = _get_program()
    in_maps = make_in_maps(feat_1d, coords_1d, cell_1d, W1, b1, Wr, br, W3, b3)
    res = run_bass_kernel_spmd(nc, in_maps, core_ids=list(range(NCORES)))
    outf = np.zeros((B, N, C), np.float32)
    for core in range(NCORES):
        b = core // 2
        s = core % 2
        outf[b, s * Q:(s + 1) * Q, :] = res.results[core]["out"]
    return outf
